# revision 1
# baseline (speedup 1.0000x reference)
"""Trainium2 Bass kernel for nn_CrossAttentionLayer (B=8, N=2048, Q=256, D=1024, H=16).

Strategy: data-parallel over batch (1 sample per NeuronCore, 8 cores).
Per-core, everything is expressed as matmuls in fp32r (TF32-like, 4x faster
than fp32 on the PE) except the probability @ V leg which runs in bf16.

Host-side preprocessing (cheap numpy):
  - transpose sources/queries/weights so contraction dims land on SBUF
    partitions without any on-device transposes
  - fold the V bias through the output projection (softmax rows sum to 1):
      out = attn @ (X_v + 1 b_v^T) @ W_o^T + b_out + queries
          = attn @ X_v @ W_o^T + (b_out + W_o b_v) + queries
  - drop the K bias entirely (adds a per-query constant to scores ->
    softmax invariant)
  - fold the 1/sqrt(HD) scale and b_q into the Q projection eviction

Device phases per core:
  P0  DMA loads (sourcesT resident in fp32r)
  P1  V = sources @ W_v^T           -> bf16, heads padded with a ones column
                                       (gives softmax denominators for free)
  P2  kT = (sources @ W_k^T)^T      -> fp32r  [D, N]
  P3  qT = ((queries @ W_q^T)+b_q)/8^T -> fp32r [D, Q]
  P4  per head: scoresT = kT_h^T-slices x qT_h  [N, Q] -> exp (ACT, bf16)
      -> outT_h[65, Q] = [V_h | 1]^T @ expT (accumulated over N tiles)
      -> normalize rows 0..63 by reciprocal of row 64 (PE-broadcast)
  P5  out = attnoutT^T @ W_o^T + (queries + b_out + W_o b_v), DMA out
"""

import numpy as np
from contextlib import ExitStack

import concourse.bass as bass
import concourse.mybir as mybir
import concourse.tile as tile
from concourse import bacc
from concourse.bass_utils import run_bass_kernel_spmd

F32 = mybir.dt.float32
F32R = mybir.dt.float32r
BF16 = mybir.dt.bfloat16
AF = mybir.ActivationFunctionType

B, N, Q, D, H = 8, 2048, 256, 1024, 16
N_CORES = 8


def build(N=N, Q=Q, D=D, H=H):
    HD = D // H           # head dim (64)
    KT = D // 128         # contraction (din) tiles
    MT = D // 128         # output (dout) tiles
    NT = N // 128         # source-token tiles
    QT = Q // 128         # query-token tiles
    HPT = 128 // HD       # heads per 128-row dout tile (2)
    NCH = min(512, D)     # fp32r moving-dim chunk (<= one PSUM bank)
    CH = 4                # score n-tiles per exp chunk ([128, CH*Q] <= 2 banks)
    KBLK = min(1024, N)   # kT eviction block
    assert D % NCH == 0 and N % (CH * 128) == 0 and N % KBLK == 0 and Q <= 512

    nc = bacc.Bacc(None, target_bir_lowering=False)
    srcT = nc.declare_dram_parameter("srcT", [D, N], F32R, isOutput=False)
    qryT = nc.declare_dram_parameter("qryT", [D, Q], F32R, isOutput=False)
    wvT = nc.declare_dram_parameter("wvT", [D, D], F32R, isOutput=False)
    wkT = nc.declare_dram_parameter("wkT", [D, D], F32R, isOutput=False)
    wqT = nc.declare_dram_parameter("wqT", [D, D], F32R, isOutput=False)
    woT = nc.declare_dram_parameter("woT", [D, D], F32R, isOutput=False)
    bq = nc.declare_dram_parameter("bq", [D], F32, isOutput=False)
    resid = nc.declare_dram_parameter("resid", [Q, D], F32, isOutput=False)
    out = nc.declare_dram_parameter("out", [Q, D], F32, isOutput=True)

    with tile.TileContext(nc) as tc, ExitStack() as ctx:
        psum = ctx.enter_context(tc.tile_pool(name="psum", bufs=4, space="PSUM"))
        kt_pool = ctx.enter_context(tc.tile_pool(name="ktp", bufs=1))
        v_pool = ctx.enter_context(tc.tile_pool(name="vp", bufs=1))
        qt_pool = ctx.enter_context(tc.tile_pool(name="qtp", bufs=1))

        kt_sb = kt_pool.tile([128, MT, N], F32R)
        v_sb = v_pool.tile([128, NT, H, HD + 1], BF16)
        qt_sb = qt_pool.tile([128, MT, Q], F32R)

        with ExitStack() as pctx:
            src_pool = pctx.enter_context(tc.tile_pool(name="srcp", bufs=1))
            wbig_pool = pctx.enter_context(tc.tile_pool(name="wbig", bufs=1))
            wsm_pool = pctx.enter_context(tc.tile_pool(name="wsm", bufs=2))
            qry_pool = pctx.enter_context(tc.tile_pool(name="qryp", bufs=1))

            src_sb = src_pool.tile([128, KT, N], F32R)
            srcT_r = srcT.rearrange("(kt p) n -> kt p n", p=128)
            for k in range(KT):
                nc.sync.dma_start(out=src_sb[:, k, :], in_=srcT_r[k])

            # ---- P1: V projection -> bf16, [n, h, hd(+ones)] ----
            HPC = NCH // HD  # heads per dout chunk
            nc.vector.memset(v_sb[:, :, :, HD:HD + 1], 1.0)
            for c in range(D // NCH):
                wv_c = wbig_pool.tile([128, KT, NCH], F32R, tag="wbig")
                nc.scalar.dma_start(
                    out=wv_c,
                    in_=wvT.rearrange("(kt p) d -> p kt d", p=128)[:, :, c * NCH:(c + 1) * NCH],
                )
                for t in range(NT):
                    ps = psum.tile([128, NCH], F32, tag="ps")
                    for k in range(KT):
                        nc.tensor.matmul(
                            ps[:],
                            lhsT=src_sb[:, k, t * 128:(t + 1) * 128],
                            rhs=wv_c[:, k, :],
                            start=(k == 0), stop=(k == KT - 1),
                        )
                    nc.vector.tensor_copy(
                        out=v_sb[:, t, c * HPC:(c + 1) * HPC, 0:HD],
                        in_=ps[:].rearrange("p (h d) -> p h d", h=HPC),
                    )

            # ---- P2: K projection -> kT [dout, n] fp32r ----
            for m in range(MT):
                wk_m = wsm_pool.tile([128, KT, 128], F32R, tag="wsm")
                nc.scalar.dma_start(
                    out=wk_m,
                    in_=wkT.rearrange("(kt p) d -> p kt d", p=128)[:, :, m * 128:(m + 1) * 128],
                )
                for half in range(N // KBLK):
                    ps = psum.tile([128, KBLK], F32, tag="ps")
                    for k in range(KT):
                        for c in range(KBLK // 512):
                            nc.tensor.matmul(
                                ps[:, c * 512:(c + 1) * 512],
                                lhsT=wk_m[:, k, :],
                                rhs=src_sb[:, k, half * KBLK + c * 512: half * KBLK + (c + 1) * 512],
                                start=(k == 0), stop=(k == KT - 1),
                            )
                    nc.vector.tensor_copy(
                        out=kt_sb[:, m, half * KBLK:(half + 1) * KBLK], in_=ps
                    )

            # ---- P3: Q projection -> qT [dout, q] fp32r, (x + b_q)/sqrt(HD) ----
            qry_sb = qry_pool.tile([128, KT, Q], F32R, tag="qry")
            nc.scalar.dma_start(out=qry_sb, in_=qryT.rearrange("(kt p) q -> p kt q", p=128))
            bq_sb = qry_pool.tile([128, MT], F32, tag="bq")
            nc.scalar.dma_start(out=bq_sb, in_=bq.rearrange("(mt p) -> p mt", p=128))
            for m in range(MT):
                wq_m = wsm_pool.tile([128, KT, 128], F32R, tag="wsm")
                nc.scalar.dma_start(
                    out=wq_m,
                    in_=wqT.rearrange("(kt p) d -> p kt d", p=128)[:, :, m * 128:(m + 1) * 128],
                )
                ps = psum.tile([128, Q], F32, tag="ps")
                for k in range(KT):
                    nc.tensor.matmul(
                        ps[:], lhsT=wq_m[:, k, :], rhs=qry_sb[:, k, :],
                        start=(k == 0), stop=(k == KT - 1),
                    )
                nc.vector.tensor_scalar(
                    out=qt_sb[:, m, :], in0=ps[:],
                    scalar1=bq_sb[:, m:m + 1], scalar2=1.0 / np.sqrt(HD),
                    op0=mybir.AluOpType.add, op1=mybir.AluOpType.mult,
                )

        # ---- P4: attention per head ----
        with ExitStack() as actx:
            exp_pool = actx.enter_context(tc.tile_pool(name="expp", bufs=3))
            rc_pool = actx.enter_context(tc.tile_pool(name="rcp", bufs=2))
            ao_pool = actx.enter_context(tc.tile_pool(name="aop", bufs=1))
            one_pool = actx.enter_context(tc.tile_pool(name="onep", bufs=1))
            wo_pool = actx.enter_context(tc.tile_pool(name="wop", bufs=1))
            res_pool = actx.enter_context(tc.tile_pool(name="resp", bufs=1))
            out_pool = actx.enter_context(tc.tile_pool(name="outp", bufs=2))

            ones_f32 = one_pool.tile([1, HD], F32, tag="ones32")
            nc.vector.memset(ones_f32, 1.0)
            ones_sb = one_pool.tile([1, HD], F32R, tag="ones")
            nc.vector.tensor_copy(ones_sb, ones_f32)

            ao_sb = ao_pool.tile([128, MT, Q], F32R)

            def emit_scores(h, expt):
                mt, po = divmod(h, HPT)
                po *= HD
                for chk in range(NT // CH):
                    ps = psum.tile([128, CH, Q], F32, tag="ps", name=f"ps_s{h}_{chk}")
                    for j in range(CH):
                        nt = chk * CH + j
                        nc.tensor.matmul(
                            ps[:, j, :],
                            lhsT=kt_sb[po:po + HD, mt, nt * 128:(nt + 1) * 128],
                            rhs=qt_sb[po:po + HD, mt, :],
                            start=True, stop=True,
                        )
                    nc.scalar.activation(
                        out=expt[:, chk * CH:(chk + 1) * CH, :], in_=ps[:], func=AF.Exp
                    )

            def emit_attn(h, expt):
                mt, po = divmod(h, HPT)
                po *= HD
                pso = psum.tile([HD + 1, Q], F32, tag="ps", name=f"pso{h}")
                for nt in range(NT):
                    nc.tensor.matmul(
                        pso[:], lhsT=v_sb[:, nt, h, :], rhs=expt[:, nt, :],
                        start=(nt == 0), stop=(nt == NT - 1),
                    )
                # normalize: rows 0..HD-1 divided by row HD (the ones-column sum)
                rc32 = rc_pool.tile([1, Q], F32, tag="rc32", name=f"rc32_{h}")
                nc.vector.reciprocal(rc32, pso[HD:HD + 1, :])
                rc = rc_pool.tile([1, Q], F32R, tag="rc", name=f"rc{h}")
                nc.vector.tensor_copy(rc, rc32)
                rbp = psum.tile([HD, Q], F32, tag="ps", name=f"rbp{h}")
                nc.tensor.matmul(rbp[:], lhsT=ones_sb[:], rhs=rc[:], start=True, stop=True)
                rb = rc_pool.tile([HD, Q], F32, tag="rb", name=f"rb{h}")
                nc.vector.tensor_copy(rb, rbp)
                nc.vector.tensor_mul(ao_sb[po:po + HD, mt, :], pso[0:HD, :], rb[:])

            # software pipeline: scores/exp of head h overlap attn@V of h-1,
            # so the PE never stalls on the ACT exp round-trip
            expts = {}
            for h in range(H):
                expts[h] = exp_pool.tile([128, NT, Q], BF16, tag="exp", name=f"expt{h}")
                emit_scores(h, expts[h])
                if h > 0:
                    emit_attn(h - 1, expts[h - 1])
            emit_attn(H - 1, expts[H - 1])

            # ---- P5: output projection + residual ----
            wo_sb = wo_pool.tile([128, KT, D], F32R, tag="wo")
            nc.sync.dma_start(out=wo_sb, in_=woT.rearrange("(kt p) d -> p kt d", p=128))
            res_sb = res_pool.tile([128, QT, D], F32, tag="res")
            nc.sync.dma_start(out=res_sb, in_=resid.rearrange("(qt p) d -> p qt d", p=128))
            for qt in range(QT):
                ps = psum.tile([128, D], F32, tag="ps")
                for k in range(KT):
                    for c in range(D // NCH):
                        nc.tensor.matmul(
                            ps[:, c * NCH:(c + 1) * NCH],
                            lhsT=ao_sb[:, k, qt * 128:(qt + 1) * 128],
                            rhs=wo_sb[:, k, c * NCH:(c + 1) * NCH],
                            start=(k == 0), stop=(k == KT - 1),
                        )
                osb = out_pool.tile([128, D], F32, tag="osb")
                nc.vector.tensor_add(osb[:], ps[:], res_sb[:, qt, :])
                nc.sync.dma_start(out=out[qt * 128:(qt + 1) * 128, :], in_=osb)

    nc.finalize()
    return nc


_NC_CACHE = {}


def _get_nc():
    key = (N, Q, D, H)
    if key not in _NC_CACHE:
        _NC_CACHE[key] = build()
    return _NC_CACHE[key]


def make_in_maps(sources, queries, w_in, b_in, w_out, b_out):
    sources = np.asarray(sources, dtype=np.float32)
    queries = np.asarray(queries, dtype=np.float32)
    w_in = np.asarray(w_in, dtype=np.float32)
    b_in = np.asarray(b_in, dtype=np.float32)
    w_out = np.asarray(w_out, dtype=np.float32)
    b_out = np.asarray(b_out, dtype=np.float32)

    w_q, w_k, w_v = w_in[0:D], w_in[D:2 * D], w_in[2 * D:3 * D]
    b_q, b_v = b_in[0:D], b_in[2 * D:3 * D]
    # b_k dropped: constant shift along softmax axis
    wqT = np.ascontiguousarray(w_q.T)
    wkT = np.ascontiguousarray(w_k.T)
    wvT = np.ascontiguousarray(w_v.T)
    woT = np.ascontiguousarray(w_out.T)
    bout_eff = b_out + w_out @ b_v

    in_maps = []
    for b in range(B):
        in_maps.append({
            "srcT": np.ascontiguousarray(sources[b].T),
            "qryT": np.ascontiguousarray(queries[b].T),
            "wvT": wvT, "wkT": wkT, "wqT": wqT, "woT": woT,
            "bq": b_q,
            "resid": queries[b] + bout_eff[None, :],
        })
    return in_maps


def kernel(sources, queries, w_in, b_in, w_out, b_out, _trace=False):
    nc = _get_nc()
    in_maps = make_in_maps(sources, queries, w_in, b_in, w_out, b_out)
    res = run_bass_kernel_spmd(nc, in_maps, core_ids=list(range(N_CORES)), trace=_trace)
    out = np.stack([res.results[b]["out"] for b in range(B)], axis=0)
    if _trace:
        kernel.last_exec_time_ns = res.exec_time_ns
        kernel.last_results = res
    return out



# revision 4
# speedup vs baseline: 2.2239x; 2.2239x over previous
"""Trainium2 Bass kernel for nn_CrossAttentionLayer (B=8, N=2048, Q=256, D=1024, H=16).

Data-parallel over batch (1 sample per NeuronCore, 8 cores).

Device strategy (per core):
  - All GEMMs except scores run as fp8e4 DoubleRow matmuls (two 128-deep
    contraction planes per instruction, 0.5 cycles/row = 2x bf16). Dual-fp8
    LDWEIGHTS requires per-plane free dim in {32,64,128}.
  - Scores are bf16 "pair-packed": heads (2m, 2m+1) share one matmul with a
    block-diagonal rhs (qt halves zero-padded), giving full 128-partition
    contraction and 512-wide streams.
  - attn@V packs both heads of a pair in one [128,2,128] lhsT; the output's
    diagonal quadrants are the two heads' contributions, off-diagonal
    quadrants are ignored (free: matmul cost scales with the moving dim).
    Softmax denominators come from an all-ones [128,2,64] lhsT against the
    same exp tiles - pre-broadcast across 64 partitions, so normalization is
    one reciprocal + two multiplies on the DVE, no PE broadcast.
  - The main loop interleaves, per head-pair i: K-proj(i) chunks, scores(i)
    chunks, attn@V(i-2) - so the PE queue never drains (keeps the 2.4GHz
    p-state) while the ACT engine pipelines exp 2 pairs behind.

Host-side preprocessing:
  - weights scaled x16 before fp8 cast (keeps values in e4m3's resolved
    range); compensating 1/2048 folded into the qt eviction, 1/256 into the
    out-proj eviction; exp computed as exp(score - 2) (softmax-invariant).
  - V bias folded through the output projection; K bias dropped (softmax
    invariant); resid = queries + b_out + w_out @ b_v added at the end.
"""

import numpy as np
import ml_dtypes
from contextlib import ExitStack

import concourse.bass as bass
import concourse.mybir as mybir
import concourse.tile as tile
from concourse import bacc
from concourse.bass_utils import run_bass_kernel_spmd

F32 = mybir.dt.float32
BF16 = mybir.dt.bfloat16
F8 = mybir.dt.float8e4
AF = mybir.ActivationFunctionType
DR = mybir.MatmulPerfMode.DoubleRow

B, N, Q, D, H = 8, 2048, 256, 1024, 16
N_CORES = 8
W_SCALE = 16.0
EXP_BIAS = -2.0


def build(N=N, Q=Q, D=D, H=H):
    HD = D // H            # 64
    KT = D // 256          # 4 DoubleRow contraction steps
    NT = N // 128          # 16 source-token tiles
    MT = D // 128          # 8 pairs (2 heads of 64 dims per 128-row tile)
    QT = Q // 128          # 2
    assert Q == 256 and HD == 64

    nc = bacc.Bacc(None, target_bir_lowering=False)
    src8 = nc.declare_dram_parameter("src8", [D, N], F8, isOutput=False)
    qry8 = nc.declare_dram_parameter("qry8", [D, Q], F8, isOutput=False)
    wv8 = nc.declare_dram_parameter("wv8", [D, D], F8, isOutput=False)
    wk8 = nc.declare_dram_parameter("wk8", [D, D], F8, isOutput=False)
    wq8 = nc.declare_dram_parameter("wq8", [D, D], F8, isOutput=False)
    wo8 = nc.declare_dram_parameter("wo8", [D, D], F8, isOutput=False)
    bq16 = nc.declare_dram_parameter("bq16", [D], F32, isOutput=False)
    resid = nc.declare_dram_parameter("resid", [Q, D], F32, isOutput=False)
    out = nc.declare_dram_parameter("out", [Q, D], F32, isOutput=True)

    def wre(w):
        return w.rearrange("(kt two p) d -> p kt two d", two=2, p=128)

    with tile.TileContext(nc) as tc, ExitStack() as ctx:
        ps_pool = ctx.enter_context(tc.tile_pool(name="psA", bufs=2, space="PSUM"))
        ss_pool = ctx.enter_context(tc.tile_pool(name="psS", bufs=2, space="PSUM"))
        po_pool = ctx.enter_context(tc.tile_pool(name="psO", bufs=1, space="PSUM"))
        dn_pool = ctx.enter_context(tc.tile_pool(name="psD", bufs=1, space="PSUM"))
        res_pool = ctx.enter_context(tc.tile_pool(name="res", bufs=1))
        exp_pool = ctx.enter_context(tc.tile_pool(name="expp", bufs=3))
        nrm_pool = ctx.enter_context(tc.tile_pool(name="nrm", bufs=2))
        osb_pool = ctx.enter_context(tc.tile_pool(name="osb", bufs=2))

        # ---- resident SBUF tensors ----
        src_sb = res_pool.tile([128, KT, 2, N], F8, tag="src")
        qry_sb = res_pool.tile([128, KT, 2, Q], F8, tag="qry")
        wq_sb = res_pool.tile([128, KT, 2, D], F8, tag="wq")
        wk_sb = res_pool.tile([128, KT, 2, D], F8, tag="wk")
        wv_sb = res_pool.tile([128, KT, 2, D], F8, tag="wv")
        wo_sb = res_pool.tile([128, KT, 2, D], F8, tag="wo")
        kt_sb = res_pool.tile([128, MT, N], BF16, tag="kt")
        v2_sb = res_pool.tile([128, NT, MT, 128], F8, tag="v2")
        qt2_sb = res_pool.tile([128, MT, 2 * Q], BF16, tag="qt2")
        ao_sb = res_pool.tile([128, MT, Q], F8, tag="ao")
        bq_sb = res_pool.tile([128, MT], F32, tag="bq")
        res_sb = res_pool.tile([128, QT, D], F32, tag="res")
        ones8_sb = res_pool.tile([128, 2, HD], F8, tag="ones8")
        ebias_sb = res_pool.tile([128, 1], F32, tag="ebias")

        # ---- DMA (scalar queue: weights; sync queue: activations) ----
        nc.scalar.dma_start(out=bq_sb, in_=bq16.rearrange("(mt p) -> p mt", p=128))
        nc.scalar.dma_start(out=wq_sb, in_=wre(wq8))
        nc.scalar.dma_start(out=wv_sb, in_=wre(wv8))
        nc.scalar.dma_start(out=wk_sb, in_=wre(wk8))
        nc.scalar.dma_start(out=wo_sb, in_=wre(wo8))
        nc.sync.dma_start(
            out=qry_sb, in_=qry8.rearrange("(kt two p) q -> p kt two q", two=2, p=128)
        )
        src_r = src8.rearrange("(kt two p) n -> p kt two n", two=2, p=128)
        for h_ in range(2):
            nc.sync.dma_start(
                out=src_sb[:, :, :, h_ * (N // 2):(h_ + 1) * (N // 2)],
                in_=src_r[:, :, :, h_ * (N // 2):(h_ + 1) * (N // 2)],
            )
        nc.sync.dma_start(out=res_sb, in_=resid.rearrange("(qt p) d -> p qt d", p=128))

        nc.vector.memset(qt2_sb, 0.0)
        nc.vector.memset(ones8_sb, 1.0)
        nc.vector.memset(ebias_sb, EXP_BIAS)

        # ---- P1: Q projection -> qt2 (pair-packed, zero-padded) ----
        for m in range(MT):
            ps = ps_pool.tile([128, 512], F32, tag="ps", name=f"psq{m}")
            for k in range(KT):
                nc.tensor.matmul(
                    ps[:, 0:Q],
                    lhsT=wq_sb[:, k, :, m * 128:(m + 1) * 128],
                    rhs=qry_sb[:, k, :, :],
                    start=(k == 0), stop=(k == KT - 1), perf_mode=DR,
                )
            # qt2 = (16*q + 16*bq)/2048 = (q + bq)/128
            nc.vector.tensor_scalar(
                out=qt2_sb[0:64, m, 0:Q], in0=ps[0:64, 0:Q],
                scalar1=bq_sb[0:64, m:m + 1], scalar2=1.0 / 2048.0,
                op0=mybir.AluOpType.add, op1=mybir.AluOpType.mult,
            )
            nc.vector.tensor_scalar(
                out=qt2_sb[64:128, m, Q:2 * Q], in0=ps[64:128, 0:Q],
                scalar1=bq_sb[64:128, m:m + 1], scalar2=1.0 / 2048.0,
                op0=mybir.AluOpType.add, op1=mybir.AluOpType.mult,
            )

        # ---- P2: V projection -> v2_sb fp8 (16*v), pair-major layout ----
        for t in range(NT):
            for c in range(2):
                ps = ps_pool.tile([128, 512], F32, tag="ps", name=f"psv{t}_{c}")
                for k in range(KT):
                    nc.tensor.matmul(
                        ps[:],
                        lhsT=src_sb[:, k, :, t * 128:(t + 1) * 128],
                        rhs=wv_sb[:, k, :, c * 512:(c + 1) * 512],
                        start=(k == 0), stop=(k == KT - 1), perf_mode=DR,
                    )
                nc.vector.tensor_copy(
                    out=v2_sb[:, t, c * 4:(c + 1) * 4, :],
                    in_=ps[:].rearrange("p (mp c) -> p mp c", mp=4),
                )

        # ---- P3: K proj + scores + exp + attn@V, interleaved per pair ----
        expts = {}

        def emit_k_chunk(m, ch):
            ps = ps_pool.tile([128, 512], F32, tag="ps", name=f"psk{m}_{ch}")
            for k in range(KT):
                nc.tensor.matmul(
                    ps[:],
                    lhsT=wk_sb[:, k, :, m * 128:(m + 1) * 128],
                    rhs=src_sb[:, k, :, ch * 512:(ch + 1) * 512],
                    start=(k == 0), stop=(k == KT - 1), perf_mode=DR,
                )
            nc.vector.tensor_copy(
                out=kt_sb[:, m, ch * 512:(ch + 1) * 512], in_=ps
            )

        def emit_score_chunk(m, c):
            # chunk c covers n tiles 2c, 2c+1 -> needs K chunk c//2 done
            ss = ss_pool.tile([128, 2, 512], F32, tag="ss", name=f"ss{m}_{c}")
            for j in range(2):
                nt = 2 * c + j
                nc.tensor.matmul(
                    ss[:, j, :],
                    lhsT=kt_sb[:, m, nt * 128:(nt + 1) * 128],
                    rhs=qt2_sb[:, m, :],
                    start=True, stop=True,
                )
            nc.scalar.activation(
                out=expts[m][:, 2 * c:2 * c + 2, :], in_=ss[:],
                func=AF.Exp, bias=ebias_sb[:],
            )

        def emit_attn_po(m, po_t):
            for j in range(NT // 2):
                nc.tensor.matmul(
                    po_t[:],
                    lhsT=v2_sb[:, 2 * j:2 * j + 2, m, :],
                    rhs=expts[m][:, 2 * j:2 * j + 2, :],
                    start=(j == 0), stop=(j == NT // 2 - 1), perf_mode=DR,
                )

        def emit_attn_dn(m, po_t, dn_t):
            for j in range(NT // 2):
                nc.tensor.matmul(
                    dn_t[:],
                    lhsT=ones8_sb[:],
                    rhs=expts[m][:, 2 * j:2 * j + 2, :],
                    start=(j == 0), stop=(j == NT // 2 - 1), perf_mode=DR,
                )
            rcp = nrm_pool.tile([HD, 2 * Q], F32, tag="rcp", name=f"rcp{m}")
            nc.vector.reciprocal_approx_fast(out=rcp, in_=dn_t[:])
            # diagonal quadrants: head 2m rows 0:64 cols 0:Q, head 2m+1
            # rows 64:128 cols Q:2Q (denominators are row-broadcast already)
            nc.vector.tensor_mul(
                ao_sb[0:HD, m, :], po_t[0:HD, 0:Q], rcp[:, 0:Q],
            )
            nc.vector.tensor_mul(
                ao_sb[HD:128, m, :], po_t[HD:128, Q:2 * Q], rcp[:, Q:2 * Q],
            )

        for i in range(MT + 2):
            if i < MT:
                m = i
                expts[m] = exp_pool.tile([128, NT, 512], F8, tag="exp", name=f"expt{m}")
                emit_k_chunk(m, 0)
                emit_k_chunk(m, 1)
                emit_score_chunk(m, 0)
                emit_score_chunk(m, 1)
                if 0 <= i - 2:
                    po_t = po_pool.tile([128, 512], F32, tag="po", name=f"po{i-2}")
                    emit_attn_po(i - 2, po_t)
                emit_k_chunk(m, 2)
                emit_score_chunk(m, 2)
                emit_score_chunk(m, 3)
                if 0 <= i - 2:
                    dn_t = dn_pool.tile([HD, 2 * Q], F32, tag="dn", name=f"dn{i-2}")
                    emit_attn_dn(i - 2, po_t, dn_t)
                    expts.pop(i - 2)
                emit_k_chunk(m, 3)
                for c in range(4, 8):
                    emit_score_chunk(m, c)
            else:
                po_t = po_pool.tile([128, 512], F32, tag="po", name=f"po{i-2}")
                emit_attn_po(i - 2, po_t)
                dn_t = dn_pool.tile([HD, 2 * Q], F32, tag="dn", name=f"dn{i-2}")
                emit_attn_dn(i - 2, po_t, dn_t)
                expts.pop(i - 2)

        # ---- P5: output projection + residual ----
        for qt in range(QT):
            for c in range(2):
                ps = ps_pool.tile([128, 512], F32, tag="ps", name=f"psf{qt}_{c}")
                for k in range(KT):
                    nc.tensor.matmul(
                        ps[:],
                        lhsT=ao_sb[:, 2 * k:2 * k + 2, qt * 128:(qt + 1) * 128],
                        rhs=wo_sb[:, k, :, c * 512:(c + 1) * 512],
                        start=(k == 0), stop=(k == KT - 1), perf_mode=DR,
                    )
                osb = osb_pool.tile([128, 512], F32, tag="osb1", name=f"osb1_{qt}_{c}")
                nc.scalar.activation(
                    out=osb, in_=ps[:], func=AF.Copy, scale=1.0 / 256.0
                )
                osb2 = osb_pool.tile([128, 512], F32, tag="osb2", name=f"osb2_{qt}_{c}")
                nc.vector.tensor_add(
                    osb2[:], osb[:], res_sb[:, qt, c * 512:(c + 1) * 512]
                )
                nc.sync.dma_start(
                    out=out[qt * 128:(qt + 1) * 128, c * 512:(c + 1) * 512], in_=osb2
                )

    nc.finalize()
    return nc


_NC_CACHE = {}


def _get_nc():
    key = (N, Q, D, H)
    if key not in _NC_CACHE:
        _NC_CACHE[key] = build()
    return _NC_CACHE[key]


def make_in_maps(sources, queries, w_in, b_in, w_out, b_out):
    FP8 = ml_dtypes.float8_e4m3
    sources = np.asarray(sources, dtype=np.float32)
    queries = np.asarray(queries, dtype=np.float32)
    w_in = np.asarray(w_in, dtype=np.float32)
    b_in = np.asarray(b_in, dtype=np.float32)
    w_out = np.asarray(w_out, dtype=np.float32)
    b_out = np.asarray(b_out, dtype=np.float32)

    w_q, w_k, w_v = w_in[0:D], w_in[D:2 * D], w_in[2 * D:3 * D]
    b_q, b_v = b_in[0:D], b_in[2 * D:3 * D]
    # b_k dropped: constant shift along softmax axis
    wq8 = np.ascontiguousarray(W_SCALE * w_q.T).astype(FP8)
    wk8 = np.ascontiguousarray(W_SCALE * w_k.T).astype(FP8)
    wv8 = np.ascontiguousarray(W_SCALE * w_v.T).astype(FP8)
    wo8 = np.ascontiguousarray(W_SCALE * w_out.T).astype(FP8)
    bq16 = (W_SCALE * b_q).astype(np.float32)
    bout_eff = b_out + w_out @ b_v

    in_maps = []
    for b in range(B):
        in_maps.append({
            "src8": np.ascontiguousarray(sources[b].T).astype(FP8),
            "qry8": np.ascontiguousarray(queries[b].T).astype(FP8),
            "wv8": wv8, "wk8": wk8, "wq8": wq8, "wo8": wo8,
            "bq16": bq16,
            "resid": queries[b] + bout_eff[None, :],
        })
    return in_maps


def kernel(sources, queries, w_in, b_in, w_out, b_out, _trace=False):
    nc = _get_nc()
    in_maps = make_in_maps(sources, queries, w_in, b_in, w_out, b_out)
    res = run_bass_kernel_spmd(nc, in_maps, core_ids=list(range(N_CORES)), trace=_trace)
    out = np.stack([res.results[b]["out"] for b in range(B)], axis=0)
    if _trace:
        kernel.last_exec_time_ns = res.exec_time_ns
        kernel.last_results = res
    return out


# revision 6
# speedup vs baseline: 2.2407x; 1.0075x over previous
"""Trainium2 Bass kernel for nn_CrossAttentionLayer (B=8, N=2048, Q=256, D=1024, H=16).

Data-parallel over batch (1 sample per NeuronCore, 8 cores).

Device strategy (per core):
  - All GEMMs except scores run as fp8e4 DoubleRow matmuls (two 128-deep
    contraction planes per instruction, 0.5 cycles/row = 2x bf16). Dual-fp8
    LDWEIGHTS requires per-plane free dim in {32,64,128}.
  - Scores are bf16 "pair-packed": heads (2m, 2m+1) share one matmul with a
    block-diagonal rhs (qt halves zero-padded), giving full 128-partition
    contraction and 512-wide streams.
  - attn@V packs both heads of a pair in one [128,2,128] lhsT; the output's
    diagonal quadrants are the two heads' contributions, off-diagonal
    quadrants are ignored (free: matmul cost scales with the moving dim).
    Softmax denominators come from an all-ones [128,2,64] lhsT against the
    same exp tiles - pre-broadcast across 64 partitions, so normalization is
    one reciprocal + two multiplies on the DVE, no PE broadcast.
  - The main loop interleaves, per head-pair i: K-proj(i) chunks, scores(i)
    chunks, attn@V(i-2) - so the PE queue never drains (keeps the 2.4GHz
    p-state) while the ACT engine pipelines exp 2 pairs behind.

Host-side preprocessing:
  - weights scaled x16 before fp8 cast (keeps values in e4m3's resolved
    range); compensating 1/2048 folded into the qt eviction, 1/256 into the
    out-proj eviction; exp computed as exp(score - 2) (softmax-invariant).
  - V bias folded through the output projection; K bias dropped (softmax
    invariant); resid = queries + b_out + w_out @ b_v added at the end.
"""

import numpy as np
import ml_dtypes
from contextlib import ExitStack

import concourse.bass as bass
import concourse.mybir as mybir
import concourse.tile as tile
from concourse import bacc
from concourse.bass_utils import run_bass_kernel_spmd

F32 = mybir.dt.float32
BF16 = mybir.dt.bfloat16
F8 = mybir.dt.float8e4
AF = mybir.ActivationFunctionType
DR = mybir.MatmulPerfMode.DoubleRow

B, N, Q, D, H = 8, 2048, 256, 1024, 16
N_CORES = 8
W_SCALE = 16.0
EXP_BIAS = -2.0


def build(N=N, Q=Q, D=D, H=H):
    HD = D // H            # 64
    KT = D // 256          # 4 DoubleRow contraction steps
    NT = N // 128          # 16 source-token tiles
    MT = D // 128          # 8 pairs (2 heads of 64 dims per 128-row tile)
    QT = Q // 128          # 2
    assert Q == 256 and HD == 64

    nc = bacc.Bacc(None, target_bir_lowering=False)
    src8 = nc.declare_dram_parameter("src8", [D, N], F8, isOutput=False)
    qry8 = nc.declare_dram_parameter("qry8", [D, Q], F8, isOutput=False)
    wv8 = nc.declare_dram_parameter("wv8", [D, D], F8, isOutput=False)
    wk8 = nc.declare_dram_parameter("wk8", [D, D], F8, isOutput=False)
    wq8 = nc.declare_dram_parameter("wq8", [D, D], F8, isOutput=False)
    wo8 = nc.declare_dram_parameter("wo8", [D, D], F8, isOutput=False)
    bq16 = nc.declare_dram_parameter("bq16", [D], F32, isOutput=False)
    resid = nc.declare_dram_parameter("resid", [Q, D], F32, isOutput=False)
    out = nc.declare_dram_parameter("out", [Q, D], F32, isOutput=True)

    def wre(w):
        return w.rearrange("(kt two p) d -> p kt two d", two=2, p=128)

    with tile.TileContext(nc) as tc, ExitStack() as ctx:
        ps_pool = ctx.enter_context(tc.tile_pool(name="psA", bufs=2, space="PSUM"))
        ss_pool = ctx.enter_context(tc.tile_pool(name="psS", bufs=2, space="PSUM"))
        po_pool = ctx.enter_context(tc.tile_pool(name="psO", bufs=1, space="PSUM"))
        dn_pool = ctx.enter_context(tc.tile_pool(name="psD", bufs=1, space="PSUM"))
        res_pool = ctx.enter_context(tc.tile_pool(name="res", bufs=1))
        exp_pool = ctx.enter_context(tc.tile_pool(name="expp", bufs=3))
        nrm_pool = ctx.enter_context(tc.tile_pool(name="nrm", bufs=2))
        osb_pool = ctx.enter_context(tc.tile_pool(name="osb", bufs=2))

        # ---- resident SBUF tensors ----
        src_sb = res_pool.tile([128, KT, 2, N], F8, tag="src")
        qry_sb = res_pool.tile([128, KT, 2, Q], F8, tag="qry")
        wq_sb = res_pool.tile([128, KT, 2, D], F8, tag="wq")
        wk_sb = res_pool.tile([128, KT, 2, D], F8, tag="wk")
        wv_sb = res_pool.tile([128, KT, 2, D], F8, tag="wv")
        wo_sb = res_pool.tile([128, KT, 2, D], F8, tag="wo")
        kt_sb = res_pool.tile([128, MT, N], BF16, tag="kt")
        v2_sb = res_pool.tile([128, NT, MT, 128], F8, tag="v2")
        qt2_sb = res_pool.tile([128, MT, 2 * Q], BF16, tag="qt2")
        ao_sb = res_pool.tile([128, MT, Q], F8, tag="ao")
        bq_sb = res_pool.tile([128, MT], F32, tag="bq")
        res_sb = res_pool.tile([128, QT, D], F32, tag="res")
        ones8_sb = res_pool.tile([128, 2, HD], F8, tag="ones8")
        ebias_sb = res_pool.tile([128, 1], F32, tag="ebias")

        # ---- DMA spread across engine queues so compute starts early:
        # scalar: wq (first need) + wo (last need); vector: wv; gpsimd: wk+bq;
        # sync: qry, src (4 n-chunks so V proj starts after the first), resid.
        nc.scalar.dma_start(out=wq_sb, in_=wre(wq8))
        nc.scalar.dma_start(out=wv_sb, in_=wre(wv8))
        nc.scalar.dma_start(out=wo_sb, in_=wre(wo8))
        nc.gpsimd.dma_start(out=wk_sb, in_=wre(wk8))
        nc.gpsimd.dma_start(out=bq_sb, in_=bq16.rearrange("(mt p) -> p mt", p=128))
        nc.sync.dma_start(
            out=qry_sb, in_=qry8.rearrange("(kt two p) q -> p kt two q", two=2, p=128)
        )
        src_r = src8.rearrange("(kt two p) n -> p kt two n", two=2, p=128)
        for h_ in range(4):
            nc.sync.dma_start(
                out=src_sb[:, :, :, h_ * (N // 4):(h_ + 1) * (N // 4)],
                in_=src_r[:, :, :, h_ * (N // 4):(h_ + 1) * (N // 4)],
            )
        nc.sync.dma_start(out=res_sb, in_=resid.rearrange("(qt p) d -> p qt d", p=128))

        nc.vector.memset(qt2_sb, 0.0)
        nc.vector.memset(ones8_sb, 1.0)
        nc.vector.memset(ebias_sb, EXP_BIAS)

        # ---- P1: Q projection -> qt2 (pair-packed, zero-padded) ----
        for m in range(MT):
            ps = ps_pool.tile([128, 512], F32, tag="ps", name=f"psq{m}")
            for k in range(KT):
                nc.tensor.matmul(
                    ps[:, 0:Q],
                    lhsT=wq_sb[:, k, :, m * 128:(m + 1) * 128],
                    rhs=qry_sb[:, k, :, :],
                    start=(k == 0), stop=(k == KT - 1), perf_mode=DR,
                )
            # qt2 = (16*q + 16*bq)/2048 = (q + bq)/128
            nc.vector.tensor_scalar(
                out=qt2_sb[0:64, m, 0:Q], in0=ps[0:64, 0:Q],
                scalar1=bq_sb[0:64, m:m + 1], scalar2=1.0 / 2048.0,
                op0=mybir.AluOpType.add, op1=mybir.AluOpType.mult,
            )
            nc.vector.tensor_scalar(
                out=qt2_sb[64:128, m, Q:2 * Q], in0=ps[64:128, 0:Q],
                scalar1=bq_sb[64:128, m:m + 1], scalar2=1.0 / 2048.0,
                op0=mybir.AluOpType.add, op1=mybir.AluOpType.mult,
            )

        # ---- P2: V projection -> v2_sb fp8 (16*v), pair-major layout ----
        for t in range(NT):
            for c in range(2):
                ps = ps_pool.tile([128, 512], F32, tag="ps", name=f"psv{t}_{c}")
                for k in range(KT):
                    nc.tensor.matmul(
                        ps[:],
                        lhsT=src_sb[:, k, :, t * 128:(t + 1) * 128],
                        rhs=wv_sb[:, k, :, c * 512:(c + 1) * 512],
                        start=(k == 0), stop=(k == KT - 1), perf_mode=DR,
                    )
                nc.vector.tensor_copy(
                    out=v2_sb[:, t, c * 4:(c + 1) * 4, :],
                    in_=ps[:].rearrange("p (mp c) -> p mp c", mp=4),
                )

        # ---- P3: K proj + scores + exp + attn@V, interleaved per pair ----
        expts = {}

        def emit_k_chunk(m, ch):
            ps = ps_pool.tile([128, 512], F32, tag="ps", name=f"psk{m}_{ch}")
            for k in range(KT):
                nc.tensor.matmul(
                    ps[:],
                    lhsT=wk_sb[:, k, :, m * 128:(m + 1) * 128],
                    rhs=src_sb[:, k, :, ch * 512:(ch + 1) * 512],
                    start=(k == 0), stop=(k == KT - 1), perf_mode=DR,
                )
            nc.vector.tensor_copy(
                out=kt_sb[:, m, ch * 512:(ch + 1) * 512], in_=ps
            )

        def emit_score_chunk(m, c):
            # chunk c covers n tiles 2c, 2c+1 -> needs K chunk c//2 done
            ss = ss_pool.tile([128, 2, 512], F32, tag="ss", name=f"ss{m}_{c}")
            for j in range(2):
                nt = 2 * c + j
                nc.tensor.matmul(
                    ss[:, j, :],
                    lhsT=kt_sb[:, m, nt * 128:(nt + 1) * 128],
                    rhs=qt2_sb[:, m, :],
                    start=True, stop=True,
                )
            nc.scalar.activation(
                out=expts[m][:, 2 * c:2 * c + 2, :], in_=ss[:],
                func=AF.Exp, bias=ebias_sb[:],
            )

        def emit_attn_po(m, po_t):
            for j in range(NT // 2):
                nc.tensor.matmul(
                    po_t[:],
                    lhsT=v2_sb[:, 2 * j:2 * j + 2, m, :],
                    rhs=expts[m][:, 2 * j:2 * j + 2, :],
                    start=(j == 0), stop=(j == NT // 2 - 1), perf_mode=DR,
                )

        def emit_attn_dn(m, po_t, dn_t):
            for j in range(NT // 2):
                nc.tensor.matmul(
                    dn_t[:],
                    lhsT=ones8_sb[:],
                    rhs=expts[m][:, 2 * j:2 * j + 2, :],
                    start=(j == 0), stop=(j == NT // 2 - 1), perf_mode=DR,
                )
            rcp = nrm_pool.tile([HD, 2 * Q], F32, tag="rcp", name=f"rcp{m}")
            nc.vector.reciprocal_approx_fast(out=rcp, in_=dn_t[:])
            # diagonal quadrants: head 2m rows 0:64 cols 0:Q, head 2m+1
            # rows 64:128 cols Q:2Q (denominators are row-broadcast already)
            nc.vector.tensor_mul(
                ao_sb[0:HD, m, :], po_t[0:HD, 0:Q], rcp[:, 0:Q],
            )
            nc.vector.tensor_mul(
                ao_sb[HD:128, m, :], po_t[HD:128, Q:2 * Q], rcp[:, Q:2 * Q],
            )

        for i in range(MT + 2):
            if i < MT:
                m = i
                expts[m] = exp_pool.tile([128, NT, 512], F8, tag="exp", name=f"expt{m}")
                emit_k_chunk(m, 0)
                emit_k_chunk(m, 1)
                emit_score_chunk(m, 0)
                emit_score_chunk(m, 1)
                if 0 <= i - 2:
                    po_t = po_pool.tile([128, 512], F32, tag="po", name=f"po{i-2}")
                    emit_attn_po(i - 2, po_t)
                emit_k_chunk(m, 2)
                emit_score_chunk(m, 2)
                emit_score_chunk(m, 3)
                if 0 <= i - 2:
                    dn_t = dn_pool.tile([HD, 2 * Q], F32, tag="dn", name=f"dn{i-2}")
                    emit_attn_dn(i - 2, po_t, dn_t)
                    expts.pop(i - 2)
                emit_k_chunk(m, 3)
                for c in range(4, 8):
                    emit_score_chunk(m, c)
            else:
                po_t = po_pool.tile([128, 512], F32, tag="po", name=f"po{i-2}")
                emit_attn_po(i - 2, po_t)
                dn_t = dn_pool.tile([HD, 2 * Q], F32, tag="dn", name=f"dn{i-2}")
                emit_attn_dn(i - 2, po_t, dn_t)
                expts.pop(i - 2)

        # ---- P5: output projection + residual ----
        for qt in range(QT):
            for c in range(2):
                ps = ps_pool.tile([128, 512], F32, tag="ps", name=f"psf{qt}_{c}")
                for k in range(KT):
                    nc.tensor.matmul(
                        ps[:],
                        lhsT=ao_sb[:, 2 * k:2 * k + 2, qt * 128:(qt + 1) * 128],
                        rhs=wo_sb[:, k, :, c * 512:(c + 1) * 512],
                        start=(k == 0), stop=(k == KT - 1), perf_mode=DR,
                    )
                osb = osb_pool.tile([128, 512], F32, tag="osb1", name=f"osb1_{qt}_{c}")
                nc.scalar.activation(
                    out=osb, in_=ps[:], func=AF.Copy, scale=1.0 / 256.0
                )
                osb2 = osb_pool.tile([128, 512], F32, tag="osb2", name=f"osb2_{qt}_{c}")
                nc.vector.tensor_add(
                    osb2[:], osb[:], res_sb[:, qt, c * 512:(c + 1) * 512]
                )
                nc.sync.dma_start(
                    out=out[qt * 128:(qt + 1) * 128, c * 512:(c + 1) * 512], in_=osb2
                )

    nc.finalize()
    return nc


_NC_CACHE = {}


def _get_nc():
    key = (N, Q, D, H)
    if key not in _NC_CACHE:
        _NC_CACHE[key] = build()
    return _NC_CACHE[key]


def make_in_maps(sources, queries, w_in, b_in, w_out, b_out):
    FP8 = ml_dtypes.float8_e4m3
    sources = np.asarray(sources, dtype=np.float32)
    queries = np.asarray(queries, dtype=np.float32)
    w_in = np.asarray(w_in, dtype=np.float32)
    b_in = np.asarray(b_in, dtype=np.float32)
    w_out = np.asarray(w_out, dtype=np.float32)
    b_out = np.asarray(b_out, dtype=np.float32)

    w_q, w_k, w_v = w_in[0:D], w_in[D:2 * D], w_in[2 * D:3 * D]
    b_q, b_v = b_in[0:D], b_in[2 * D:3 * D]
    # b_k dropped: constant shift along softmax axis
    wq8 = np.ascontiguousarray(W_SCALE * w_q.T).astype(FP8)
    wk8 = np.ascontiguousarray(W_SCALE * w_k.T).astype(FP8)
    wv8 = np.ascontiguousarray(W_SCALE * w_v.T).astype(FP8)
    wo8 = np.ascontiguousarray(W_SCALE * w_out.T).astype(FP8)
    bq16 = (W_SCALE * b_q).astype(np.float32)
    bout_eff = b_out + w_out @ b_v

    in_maps = []
    for b in range(B):
        in_maps.append({
            "src8": np.ascontiguousarray(sources[b].T).astype(FP8),
            "qry8": np.ascontiguousarray(queries[b].T).astype(FP8),
            "wv8": wv8, "wk8": wk8, "wq8": wq8, "wo8": wo8,
            "bq16": bq16,
            "resid": queries[b] + bout_eff[None, :],
        })
    return in_maps


def kernel(sources, queries, w_in, b_in, w_out, b_out, _trace=False):
    nc = _get_nc()
    in_maps = make_in_maps(sources, queries, w_in, b_in, w_out, b_out)
    res = run_bass_kernel_spmd(nc, in_maps, core_ids=list(range(N_CORES)), trace=_trace)
    out = np.stack([res.results[b]["out"] for b in range(B)], axis=0)
    if _trace:
        kernel.last_exec_time_ns = res.exec_time_ns
        kernel.last_results = res
    return out


# revision 7
# speedup vs baseline: 2.2775x; 1.0164x over previous
"""Trainium2 Bass kernel for nn_CrossAttentionLayer (B=8, N=2048, Q=256, D=1024, H=16).

Data-parallel over batch (1 sample per NeuronCore, 8 cores).

Device strategy (per core):
  - All GEMMs except scores run as fp8e4 DoubleRow matmuls (two 128-deep
    contraction planes per instruction, 0.5 cycles/row = 2x bf16). Dual-fp8
    LDWEIGHTS requires per-plane free dim in {32,64,128}.
  - Scores are bf16 "pair-packed": heads (2m, 2m+1) share one matmul with a
    block-diagonal rhs (qt halves zero-padded), giving full 128-partition
    contraction and 512-wide streams.
  - attn@V packs both heads of a pair in one [128,2,128] lhsT; the output's
    diagonal quadrants are the two heads' contributions, off-diagonal
    quadrants are ignored (free: matmul cost scales with the moving dim).
    Softmax denominators come from an all-ones [128,2,64] lhsT against the
    same exp tiles - pre-broadcast across 64 partitions, so normalization is
    one reciprocal + two multiplies on the DVE, no PE broadcast.
  - The main loop interleaves, per head-pair i: K-proj(i) chunks, scores(i)
    chunks, attn@V(i-2) - so the PE queue never drains (keeps the 2.4GHz
    p-state) while the ACT engine pipelines exp 2 pairs behind.

Host-side preprocessing:
  - weights scaled x16 before fp8 cast (keeps values in e4m3's resolved
    range); compensating 1/2048 folded into the qt eviction, 1/256 into the
    out-proj eviction; exp computed as exp(score - 2) (softmax-invariant).
  - V bias folded through the output projection; K bias dropped (softmax
    invariant); resid = queries + b_out + w_out @ b_v added at the end.
"""

import numpy as np
import ml_dtypes
from contextlib import ExitStack

import concourse.bass as bass
import concourse.mybir as mybir
import concourse.tile as tile
from concourse import bacc
from concourse.bass_utils import run_bass_kernel_spmd

F32 = mybir.dt.float32
BF16 = mybir.dt.bfloat16
F8 = mybir.dt.float8e4
AF = mybir.ActivationFunctionType
DR = mybir.MatmulPerfMode.DoubleRow

B, N, Q, D, H = 8, 2048, 256, 1024, 16
N_CORES = 8
W_SCALE = 16.0
EXP_BIAS = -2.0


def build(N=N, Q=Q, D=D, H=H):
    HD = D // H            # 64
    KT = D // 256          # 4 DoubleRow contraction steps
    NT = N // 128          # 16 source-token tiles
    MT = D // 128          # 8 pairs (2 heads of 64 dims per 128-row tile)
    QT = Q // 128          # 2
    assert Q == 256 and HD == 64

    nc = bacc.Bacc(None, target_bir_lowering=False)
    src8 = nc.declare_dram_parameter("src8", [128, KT, 2, N], F8, isOutput=False)
    qry8 = nc.declare_dram_parameter("qry8", [128, KT, 2, Q], F8, isOutput=False)
    wv8 = nc.declare_dram_parameter("wv8", [128, KT, 2, D], F8, isOutput=False)
    wk8 = nc.declare_dram_parameter("wk8", [128, KT, 2, D], F8, isOutput=False)
    wq8 = nc.declare_dram_parameter("wq8", [128, KT, 2, D], F8, isOutput=False)
    wo8 = nc.declare_dram_parameter("wo8", [128, KT, 2, D], F8, isOutput=False)
    bq16 = nc.declare_dram_parameter("bq16", [128, MT], F32, isOutput=False)
    resid = nc.declare_dram_parameter("resid", [128, QT, D], F32, isOutput=False)
    out = nc.declare_dram_parameter("out", [Q, D], F32, isOutput=True)

    with tile.TileContext(nc) as tc, ExitStack() as ctx:
        ps_pool = ctx.enter_context(tc.tile_pool(name="psA", bufs=2, space="PSUM"))
        ss_pool = ctx.enter_context(tc.tile_pool(name="psS", bufs=2, space="PSUM"))
        po_pool = ctx.enter_context(tc.tile_pool(name="psO", bufs=1, space="PSUM"))
        dn_pool = ctx.enter_context(tc.tile_pool(name="psD", bufs=1, space="PSUM"))
        res_pool = ctx.enter_context(tc.tile_pool(name="res", bufs=1))
        exp_pool = ctx.enter_context(tc.tile_pool(name="expp", bufs=3))
        nrm_pool = ctx.enter_context(tc.tile_pool(name="nrm", bufs=2))
        osb_pool = ctx.enter_context(tc.tile_pool(name="osb", bufs=2))

        # ---- resident SBUF tensors ----
        src_sb = res_pool.tile([128, KT, 2, N], F8, tag="src")
        qry_sb = res_pool.tile([128, KT, 2, Q], F8, tag="qry")
        wq_sb = res_pool.tile([128, KT, 2, D], F8, tag="wq")
        wk_sb = res_pool.tile([128, KT, 2, D], F8, tag="wk")
        wv_sb = res_pool.tile([128, KT, 2, D], F8, tag="wv")
        wo_sb = res_pool.tile([128, KT, 2, D], F8, tag="wo")
        kt_sb = res_pool.tile([128, MT, N], BF16, tag="kt")
        v2_sb = res_pool.tile([128, NT, MT, 128], F8, tag="v2")
        qt2_sb = res_pool.tile([128, MT, 2 * Q], BF16, tag="qt2")
        ao_sb = res_pool.tile([128, MT, Q], F8, tag="ao")
        bq_sb = res_pool.tile([128, MT], F32, tag="bq")
        res_sb = res_pool.tile([128, QT, D], F32, tag="res")
        ones8_sb = res_pool.tile([128, 2, HD], F8, tag="ones8")
        ebias_sb = res_pool.tile([128, 1], F32, tag="ebias")

        # ---- DMA spread across engine queues so compute starts early:
        # scalar: wq (first need) + wo (last need); vector: wv; gpsimd: wk+bq;
        # sync: qry, src (4 n-chunks so V proj starts after the first), resid.
        nc.scalar.dma_start(out=wq_sb, in_=wq8[:, :, :, :])
        nc.scalar.dma_start(out=wv_sb, in_=wv8[:, :, :, :])
        nc.scalar.dma_start(out=wo_sb, in_=wo8[:, :, :, :])
        nc.gpsimd.dma_start(out=bq_sb, in_=bq16[:, :])
        nc.gpsimd.dma_start(out=wk_sb, in_=wk8[:, :, :, :])
        nc.sync.dma_start(out=qry_sb, in_=qry8[:, :, :, :])
        nc.sync.dma_start(out=src_sb, in_=src8[:, :, :, :])
        nc.sync.dma_start(out=res_sb, in_=resid[:, :, :])

        nc.vector.memset(qt2_sb, 0.0)
        nc.vector.memset(ones8_sb, 1.0)
        nc.vector.memset(ebias_sb, EXP_BIAS)

        # ---- P1: Q projection -> qt2 (pair-packed, zero-padded) ----
        for m in range(MT):
            ps = ps_pool.tile([128, 512], F32, tag="ps", name=f"psq{m}")
            for k in range(KT):
                nc.tensor.matmul(
                    ps[:, 0:Q],
                    lhsT=wq_sb[:, k, :, m * 128:(m + 1) * 128],
                    rhs=qry_sb[:, k, :, :],
                    start=(k == 0), stop=(k == KT - 1), perf_mode=DR,
                )
            # qt2 = (16*q + 16*bq)/2048 = (q + bq)/128
            nc.vector.tensor_scalar(
                out=qt2_sb[0:64, m, 0:Q], in0=ps[0:64, 0:Q],
                scalar1=bq_sb[0:64, m:m + 1], scalar2=1.0 / 2048.0,
                op0=mybir.AluOpType.add, op1=mybir.AluOpType.mult,
            )
            nc.vector.tensor_scalar(
                out=qt2_sb[64:128, m, Q:2 * Q], in0=ps[64:128, 0:Q],
                scalar1=bq_sb[64:128, m:m + 1], scalar2=1.0 / 2048.0,
                op0=mybir.AluOpType.add, op1=mybir.AluOpType.mult,
            )

        # ---- P2: V projection -> v2_sb fp8 (16*v), pair-major layout ----
        for t in range(NT):
            for c in range(2):
                ps = ps_pool.tile([128, 512], F32, tag="ps", name=f"psv{t}_{c}")
                for k in range(KT):
                    nc.tensor.matmul(
                        ps[:],
                        lhsT=src_sb[:, k, :, t * 128:(t + 1) * 128],
                        rhs=wv_sb[:, k, :, c * 512:(c + 1) * 512],
                        start=(k == 0), stop=(k == KT - 1), perf_mode=DR,
                    )
                nc.vector.tensor_copy(
                    out=v2_sb[:, t, c * 4:(c + 1) * 4, :],
                    in_=ps[:].rearrange("p (mp c) -> p mp c", mp=4),
                )

        # ---- P3: K proj + scores + exp + attn@V, interleaved per pair ----
        expts = {}

        def emit_k_chunk(m, ch):
            ps = ps_pool.tile([128, 512], F32, tag="ps", name=f"psk{m}_{ch}")
            for k in range(KT):
                nc.tensor.matmul(
                    ps[:],
                    lhsT=wk_sb[:, k, :, m * 128:(m + 1) * 128],
                    rhs=src_sb[:, k, :, ch * 512:(ch + 1) * 512],
                    start=(k == 0), stop=(k == KT - 1), perf_mode=DR,
                )
            nc.vector.tensor_copy(
                out=kt_sb[:, m, ch * 512:(ch + 1) * 512], in_=ps
            )

        def emit_score_chunk(m, c):
            # chunk c covers n tiles 2c, 2c+1 -> needs K chunk c//2 done
            ss = ss_pool.tile([128, 2, 512], F32, tag="ss", name=f"ss{m}_{c}")
            for j in range(2):
                nt = 2 * c + j
                nc.tensor.matmul(
                    ss[:, j, :],
                    lhsT=kt_sb[:, m, nt * 128:(nt + 1) * 128],
                    rhs=qt2_sb[:, m, :],
                    start=True, stop=True,
                )
            nc.scalar.activation(
                out=expts[m][:, 2 * c:2 * c + 2, :], in_=ss[:],
                func=AF.Exp, bias=ebias_sb[:],
            )

        def emit_attn_po(m, po_t):
            for j in range(NT // 2):
                nc.tensor.matmul(
                    po_t[:],
                    lhsT=v2_sb[:, 2 * j:2 * j + 2, m, :],
                    rhs=expts[m][:, 2 * j:2 * j + 2, :],
                    start=(j == 0), stop=(j == NT // 2 - 1), perf_mode=DR,
                )

        def emit_attn_dn(m, po_t, dn_t):
            for j in range(NT // 2):
                nc.tensor.matmul(
                    dn_t[:],
                    lhsT=ones8_sb[:],
                    rhs=expts[m][:, 2 * j:2 * j + 2, :],
                    start=(j == 0), stop=(j == NT // 2 - 1), perf_mode=DR,
                )
            rcp = nrm_pool.tile([HD, 2 * Q], F32, tag="rcp", name=f"rcp{m}")
            nc.vector.reciprocal_approx_fast(out=rcp, in_=dn_t[:])
            # diagonal quadrants: head 2m rows 0:64 cols 0:Q, head 2m+1
            # rows 64:128 cols Q:2Q (denominators are row-broadcast already)
            nc.vector.tensor_mul(
                ao_sb[0:HD, m, :], po_t[0:HD, 0:Q], rcp[:, 0:Q],
            )
            nc.vector.tensor_mul(
                ao_sb[HD:128, m, :], po_t[HD:128, Q:2 * Q], rcp[:, Q:2 * Q],
            )

        for i in range(MT + 2):
            if i < MT:
                m = i
                expts[m] = exp_pool.tile([128, NT, 512], F8, tag="exp", name=f"expt{m}")
                emit_k_chunk(m, 0)
                emit_k_chunk(m, 1)
                emit_score_chunk(m, 0)
                emit_score_chunk(m, 1)
                if 0 <= i - 2:
                    po_t = po_pool.tile([128, 512], F32, tag="po", name=f"po{i-2}")
                    emit_attn_po(i - 2, po_t)
                emit_k_chunk(m, 2)
                emit_score_chunk(m, 2)
                emit_score_chunk(m, 3)
                if 0 <= i - 2:
                    dn_t = dn_pool.tile([HD, 2 * Q], F32, tag="dn", name=f"dn{i-2}")
                    emit_attn_dn(i - 2, po_t, dn_t)
                    expts.pop(i - 2)
                emit_k_chunk(m, 3)
                for c in range(4, 8):
                    emit_score_chunk(m, c)
            else:
                po_t = po_pool.tile([128, 512], F32, tag="po", name=f"po{i-2}")
                emit_attn_po(i - 2, po_t)
                dn_t = dn_pool.tile([HD, 2 * Q], F32, tag="dn", name=f"dn{i-2}")
                emit_attn_dn(i - 2, po_t, dn_t)
                expts.pop(i - 2)

        # ---- P5: output projection + residual ----
        for qt in range(QT):
            for c in range(2):
                ps = ps_pool.tile([128, 512], F32, tag="ps", name=f"psf{qt}_{c}")
                for k in range(KT):
                    nc.tensor.matmul(
                        ps[:],
                        lhsT=ao_sb[:, 2 * k:2 * k + 2, qt * 128:(qt + 1) * 128],
                        rhs=wo_sb[:, k, :, c * 512:(c + 1) * 512],
                        start=(k == 0), stop=(k == KT - 1), perf_mode=DR,
                    )
                osb = osb_pool.tile([128, 512], F32, tag="osb1", name=f"osb1_{qt}_{c}")
                nc.scalar.activation(
                    out=osb, in_=ps[:], func=AF.Copy, scale=1.0 / 256.0
                )
                osb2 = osb_pool.tile([128, 512], F32, tag="osb2", name=f"osb2_{qt}_{c}")
                nc.vector.tensor_add(
                    osb2[:], osb[:], res_sb[:, qt, c * 512:(c + 1) * 512]
                )
                nc.sync.dma_start(
                    out=out[qt * 128:(qt + 1) * 128, c * 512:(c + 1) * 512], in_=osb2
                )

    nc.finalize()
    return nc


_NC_CACHE = {}


def _get_nc():
    key = (N, Q, D, H)
    if key not in _NC_CACHE:
        _NC_CACHE[key] = build()
    return _NC_CACHE[key]


def make_in_maps(sources, queries, w_in, b_in, w_out, b_out):
    FP8 = ml_dtypes.float8_e4m3
    sources = np.asarray(sources, dtype=np.float32)
    queries = np.asarray(queries, dtype=np.float32)
    w_in = np.asarray(w_in, dtype=np.float32)
    b_in = np.asarray(b_in, dtype=np.float32)
    w_out = np.asarray(w_out, dtype=np.float32)
    b_out = np.asarray(b_out, dtype=np.float32)

    w_q, w_k, w_v = w_in[0:D], w_in[D:2 * D], w_in[2 * D:3 * D]
    b_q, b_v = b_in[0:D], b_in[2 * D:3 * D]
    # b_k dropped: constant shift along softmax axis
    def pre(a):
        # [din, X] -> [128, KT, 2, X] with din = kt*256 + two*128 + p
        return np.ascontiguousarray(
            a.reshape(D // 256, 2, 128, -1).transpose(2, 0, 1, 3))

    wq8 = pre((W_SCALE * w_q.T).astype(FP8))
    wk8 = pre((W_SCALE * w_k.T).astype(FP8))
    wv8 = pre((W_SCALE * w_v.T).astype(FP8))
    wo8 = pre((W_SCALE * w_out.T).astype(FP8))
    bq16 = np.ascontiguousarray(
        (W_SCALE * b_q).astype(np.float32).reshape(D // 128, 128).T)
    bout_eff = b_out + w_out @ b_v

    in_maps = []
    for b in range(B):
        in_maps.append({
            "src8": pre(sources[b].T.astype(FP8)),
            "qry8": pre(queries[b].T.astype(FP8)),
            "wv8": wv8, "wk8": wk8, "wq8": wq8, "wo8": wo8,
            "bq16": bq16,
            "resid": np.ascontiguousarray(
                (queries[b] + bout_eff[None, :]).astype(np.float32)
                .reshape(Q // 128, 128, D).transpose(1, 0, 2)),
        })
    return in_maps


def kernel(sources, queries, w_in, b_in, w_out, b_out, _trace=False):
    nc = _get_nc()
    in_maps = make_in_maps(sources, queries, w_in, b_in, w_out, b_out)
    res = run_bass_kernel_spmd(nc, in_maps, core_ids=list(range(N_CORES)), trace=_trace)
    out = np.stack([res.results[b]["out"] for b in range(B)], axis=0)
    if _trace:
        kernel.last_exec_time_ns = res.exec_time_ns
        kernel.last_results = res
    return out
